# revision 28
# baseline (speedup 1.0000x reference)
"""Trainium2 Bass kernel for Llama-style GQA attention (B=1, S=2048, D=4096,
32 Q heads / 8 KV heads, head_dim 128, RoPE, causal mask).

Sharding: 8-way tensor-parallel over heads. Core c computes Q heads 4c..4c+3
and KV head c end-to-end (projections + RoPE + attention + its rows of wo),
producing a partial [S, D] output in bf16; the host sums the 8 partials (the
all-reduce of the row-parallel wo).

v3 design (all matmul operands bf16, PSUM accumulation fp32):
  - All four 512-wide projection groups run first; each group's RoPE (DVE)
    overlaps the next group's projections (PE), so the PE never waits on
    RoPE except ~2us at each boundary.  PSUM tiles are released after a
    SINGLE full-width read: q*sin is recomputed from q*cos via a tangent
    table (q2 = p1 * tan, numerically safe since no cancellation).
  - RoPE's even/odd interleave is folded into a column permutation of
    wq/wk; cos/tan tables are stacked [c;c] and duplicated per head-pair
    so RoPE runs 1024 wide for q.
  - Scores transposed: ST[sk, sq] = K @ Q^T.  Two single-bank score
    matmuls per (m, head-pair) share an adjacent-bank PSUM tile so exp is
    ONE wide ACT instruction.  Causal trimming: score/exp/PV/exsum touch
    only valid columns; the diagonal 128x128 triangle is zeroed with one
    multiplicative bf16 pattern.
  - Softmax denominators: DVE accumulates exp tiles (fp32), a ones-matrix
    matmul broadcasts the partition-sum to all 128 partitions, one
    custom-DVE reciprocal_approx and a multiply normalize PSUM context
    into bf16 ctx.  No serial [1,N] reciprocals, no per-tile denominator
    matmuls.
  - Output projection: wo resident, kk-outer 512-wide matmuls, po drained
    by split DVE/ACT copies to bf16 and DMA'd out.
"""

import math
import numpy as np

P = 128          # SBUF partitions / head_dim / tile edge
S = 2048         # sequence length
D = 4096         # model dim
HD = 128         # head dim
N_HEADS = 32
N_KV = 8
N_CORES = 8
NH_LOC = N_HEADS // N_CORES   # 4 local Q heads
SG = 512         # query-group width
NG = S // SG     # 4 q-position groups
KT = D // P      # 32 contraction tiles for projections
NSK = S // P     # 16 key tiles

_CACHE = {}


def _build_program():
    import concourse.tile as tile
    from concourse import bacc, mybir
    from concourse.masks import make_identity
    from contextlib import ExitStack

    f32 = mybir.dt.float32
    bf16 = mybir.dt.bfloat16
    Exp = mybir.ActivationFunctionType.Exp

    nc = bacc.Bacc()
    xt_d = nc.dram_tensor("xt", [P, NG * KT * SG], bf16, kind="ExternalInput")
    wq_d = nc.dram_tensor("wq", [P, KT * NH_LOC * HD], bf16, kind="ExternalInput")
    wk_d = nc.dram_tensor("wk", [P, KT * HD], bf16, kind="ExternalInput")
    wv_d = nc.dram_tensor("wv", [P, KT * HD], bf16, kind="ExternalInput")
    wo_d = nc.dram_tensor("wo", [P, (D // SG) * NH_LOC * SG], bf16,
                          kind="ExternalInput")
    ccp_d = nc.dram_tensor("ccp", [P, S], f32, kind="ExternalInput")
    ttp_d = nc.dram_tensor("ttp", [P, S], f32, kind="ExternalInput")
    pat_d = nc.dram_tensor("pat", [P, 2 * P], bf16, kind="ExternalInput")
    out_d = nc.dram_tensor("out", [S, D], bf16, kind="ExternalOutput")

    with ExitStack() as ctx:
        tc = ctx.enter_context(tile.TileContext(nc))
        consts = ctx.enter_context(tc.tile_pool(name="consts", bufs=1))
        kv = ctx.enter_context(tc.tile_pool(name="kv", bufs=1))
        xp = ctx.enter_context(tc.tile_pool(name="xp", bufs=5))
        qp = ctx.enter_context(tc.tile_pool(name="qp", bufs=4))
        rp = ctx.enter_context(tc.tile_pool(name="rp", bufs=2))
        ep = ctx.enter_context(tc.tile_pool(name="ep", bufs=4))
        obp = ctx.enter_context(tc.tile_pool(name="obp", bufs=3))

        # ---- resident weights / constants ----
        # staged in k-rounds so phase A(0) can start after ~1.5MB arrives;
        # wo is emitted after A(0) so it never starves the xt stream.
        wq_sb = consts.tile([P, KT * NH_LOC * HD], bf16)
        wk_sb = consts.tile([P, KT * HD], bf16)
        wv_sb = consts.tile([P, KT * HD], bf16)
        qtr = KT * NH_LOC * HD // 8
        ktr = KT * HD // 8
        for i in range(8):
            nc.scalar.dma_start(wq_sb[:, i * qtr:(i + 1) * qtr],
                                wq_d[:, i * qtr:(i + 1) * qtr])
            nc.scalar.dma_start(wk_sb[:, i * ktr:(i + 1) * ktr],
                                wk_d[:, i * ktr:(i + 1) * ktr])
            nc.scalar.dma_start(wv_sb[:, i * ktr:(i + 1) * ktr],
                                wv_d[:, i * ktr:(i + 1) * ktr])
        ccp_sb = consts.tile([P, S], f32)
        ttp_sb = consts.tile([P, S], f32)
        nc.scalar.dma_start(ccp_sb[:], ccp_d[:, :])
        nc.scalar.dma_start(ttp_sb[:], ttp_d[:, :])
        pat_sb = consts.tile([P, 2 * P], bf16)
        nc.scalar.dma_start(pat_sb[:], pat_d[:, :])

        ones_f = consts.tile([P, P], f32)
        nc.vector.memset(ones_f[:], 1.0)
        onesm = consts.tile([P, P], bf16)
        nc.vector.tensor_copy(onesm[:], ones_f[:])
        ident = consts.tile([P, P], f32)
        make_identity(nc, ident[:])
        identb = consts.tile([P, P], bf16)
        nc.vector.tensor_copy(identb[:], ident[:])

        # ---- persistent per-sequence state ----
        kT_sb = kv.tile([P, S], bf16)                 # [hd', sk]
        v_sb = kv.tile([P, S], bf16)                  # [sk%P, (sk//P)*HD+hd]
        ctx_sb = kv.tile([P, NH_LOC * S], bf16)       # [hd, h*S + sq]
        exsum = kv.tile([P, NH_LOC * SG], f32)        # [sk', h*SG + sq-in-G]
        exsum_bf = kv.tile([P, NH_LOC * SG], bf16)

        ctx3 = ctx_sb[:].rearrange("p (h c) -> p h c", h=NH_LOC)
        exs3 = exsum[:].rearrange("p (h c) -> p h c", h=NH_LOC)
        pat3 = pat_sb[:].rearrange("p (h c) -> p h c", h=2)

        ab_psum = tc.tile_pool(name="ps", bufs=2, space="PSUM")
        ps = ab_psum.__enter__()

        wo_sb = consts.tile([P, (D // SG) * NH_LOC * SG], bf16)

        # ================= phase A: projections + RoPE, all groups =========
        qts = []
        for G in range(NG):
            pq01 = ps.tile([P, 2 * SG], f32, tag="acc2", bufs=2, name="pq01")
            pq23 = ps.tile([P, 2 * SG], f32, tag="acc2", bufs=2, name="pq23")
            pkv = ps.tile([P, 2 * SG], f32, tag="stp1", bufs=1, name="pkv")
            for k4 in range(KT // 4):
                xt2 = xp.tile([P, 4 * SG], bf16, tag="xt", bufs=3, name="xt")
                blk = (G * KT + 4 * k4) * SG
                if G == 0 and k4 == 0:
                    for q in range(4):
                        nc.gpsimd.dma_start(xt2[:, q * SG:(q + 1) * SG],
                                            xt_d[:, blk + q * SG:blk + (q + 1) * SG])
                else:
                    nc.gpsimd.dma_start(xt2[:], xt_d[:, blk:blk + 4 * SG])
                for kk in (0, 1, 2, 3):
                    k = 4 * k4 + kk
                    xt = xt2[:, kk * SG:(kk + 1) * SG]
                    st_k, sp_k = (k == 0), (k == KT - 1)
                    for l in range(2):
                        nc.tensor.matmul(
                            pq01[:, l * SG:(l + 1) * SG],
                            wq_sb[:, k * SG + l * HD:k * SG + (l + 1) * HD],
                            xt, start=st_k, stop=sp_k)
                    for l in range(2, 4):
                        nc.tensor.matmul(
                            pq23[:, (l - 2) * SG:(l - 1) * SG],
                            wq_sb[:, k * SG + l * HD:k * SG + (l + 1) * HD],
                            xt, start=st_k, stop=sp_k)
                    nc.tensor.matmul(pkv[:, 0:SG],
                                     wk_sb[:, k * HD:(k + 1) * HD], xt,
                                     start=st_k, stop=sp_k)
                    nc.tensor.matmul(pkv[:, SG:2 * SG],
                                     wv_sb[:, k * HD:(k + 1) * HD], xt,
                                     start=st_k, stop=sp_k)

            if G == 0:
                # wo loads (4MB) start only now: keeps startup DMA light
                wtr = (D // SG) * NH_LOC * SG // 4
                for i in range(4):
                    nc.scalar.dma_start(wo_sb[:, i * wtr:(i + 1) * wtr],
                                        wo_d[:, i * wtr:(i + 1) * wtr])

            # ---- RoPE ----  (rows 0:64 "real" tr, 64:128 "imag" ti)
            # p1 = src*[c;c]  (the ONLY psum read -> frees the bank fast)
            # q2[0:64]=p1[64:]*tan[64:]=ti*s ; q2[64:]=p1[0:64]*tan[0:64]=tr*s
            # top = tr*c - ti*s ; bot = tr*s + ti*c   (all SBUF, base-aligned)
            qt4 = qp.tile([P, NH_LOC * SG], bf16, tag="qT", bufs=4, name="qT")
            qts.append(qt4)
            cpw = ccp_sb[:, G * SG:(G + 1) * SG]   # [c;c] for this window
            tpw = ttp_sb[:, G * SG:(G + 1) * SG]

            vt = rp.tile([P, SG], bf16, tag="vt", bufs=1)
            nc.vector.tensor_copy(vt[:], pkv[:, SG:2 * SG])
            p1k = rp.tile([P, SG], bf16, tag="p1k", bufs=1)
            nc.vector.tensor_mul(p1k[:], pkv[:, 0:SG], cpw)
            p1q = []
            for pq in (pq01, pq23):
                p1 = rp.tile([P, 2 * SG], bf16, tag="p1", bufs=2)
                nc.vector.tensor_mul(p1[:, 0:SG], pq[:, 0:SG], cpw)
                nc.vector.tensor_mul(p1[:, SG:2 * SG], pq[:, SG:2 * SG], cpw)
                p1q.append(p1)

            # v transpose on PE while DVE continues RoPE
            ptr4 = ps.tile([P, 2 * SG], bf16, tag="po2", bufs=1, name="ptr4")
            for j in range(4):
                nc.tensor.transpose(ptr4[:, j * P:(j + 1) * P],
                                    vt[:, j * P:(j + 1) * P], identb[:])

            def rope_finish(G=G, qt4=qt4, p1q=p1q, p1k=p1k, ptr4=ptr4,
                            tpw=tpw):
                for i, p1 in enumerate(p1q):
                    q2 = rp.tile([P, 2 * SG], bf16, tag="q2", bufs=1)
                    for hs in (slice(0, SG), slice(SG, 2 * SG)):
                        nc.vector.tensor_mul(q2[0:64, hs], p1[64:128, hs],
                                             tpw[64:128, :])
                        nc.vector.tensor_mul(q2[64:128, hs], p1[0:64, hs],
                                             tpw[0:64, :])
                    dst = qt4[:, 2 * i * SG:(2 * i + 2) * SG]
                    nc.vector.tensor_sub(dst[0:64, :], p1[0:64, :], q2[0:64, :])
                    nc.vector.tensor_add(dst[64:128, :], q2[64:128, :],
                                         p1[64:128, :])
                q2k = rp.tile([P, SG], bf16, tag="q2k", bufs=1)
                nc.vector.tensor_mul(q2k[0:64, :], p1k[64:128, :], tpw[64:128, :])
                nc.vector.tensor_mul(q2k[64:128, :], p1k[0:64, :], tpw[0:64, :])
                gsl = slice(G * SG, (G + 1) * SG)
                nc.vector.tensor_sub(kT_sb[0:64, gsl], p1k[0:64, :], q2k[0:64, :])
                nc.vector.tensor_add(kT_sb[64:128, gsl], q2k[64:128, :],
                                     p1k[64:128, :])
                nc.vector.tensor_copy(v_sb[:, 4 * G * HD:(4 * G + 4) * HD],
                                      ptr4[:, 0:SG])

            if G < NG - 1:
                rope_finish()
            else:
                pending_rope = rope_finish   # emit after B(0): keeps B(0)'s
                                             # patmul/exsum off the DVE queue

        # ================= phase B/C fused: attention + output proj ========
        # phase C is decomposed into single-bank pieces (4 matmuls each);
        # pieces for already-finalized groups are interleaved into phase B's
        # exp-bound iterations so the PE fills ACT-wait windows.
        cq = []

        def emit_piece(in_b):
            mt, ep = cq.pop(0)
            po = ps.tile([P, 2 * SG], f32, tag="po2", bufs=1, name="po")
            for kk in range(NH_LOC):
                for i in range(2):
                    e = 2 * ep + i
                    nc.tensor.matmul(po[:, i * SG:(i + 1) * SG],
                                     ctx3[:, kk, mt * P:(mt + 1) * P],
                                     wo_sb[:, (e * NH_LOC + kk) * SG:
                                           (e * NH_LOC + kk + 1) * SG],
                                     start=(kk == 0), stop=(kk == NH_LOC - 1))
            ob = obp.tile([P, 2 * SG], bf16, tag="ob", bufs=4)
            if in_b or (len(cq) % 2 == 0):
                nc.vector.tensor_copy(ob[:], po[:])
            else:
                nc.scalar.copy(ob[:], po[:])
            nc.sync.dma_start(out_d[mt * P:(mt + 1) * P,
                                    2 * ep * SG:(2 * ep + 2) * SG], ob[:])

        for G in range(NG):
            gsl = slice(G * SG, (G + 1) * SG)
            n_sk = 4 * (G + 1)
            qt4 = qts[G]
            cacc = [ps.tile([P, 2 * SG], f32, tag="acc2", bufs=2, name=f"cacc{p}")
                    for p in range(2)]
            for m in range(n_sk):
                j = m - 4 * G
                off = max(0, j) * P
                last = (m == n_sk - 1)
                for p in range(2):
                    stp = ps.tile([P, 2 * SG], f32, tag="stp1", bufs=1, name="stp")
                    stp3 = stp[:].rearrange("p (h c) -> p h c", h=2)
                    ex = ep.tile([P, 2 * SG], bf16, tag="ex", bufs=4, name="ex")
                    ex3 = ex[:].rearrange("p (h c) -> p h c", h=2)
                    for h in range(2):
                        hh = 2 * p + h
                        nc.tensor.matmul(
                            stp[:, h * SG + off:(h + 1) * SG],
                            kT_sb[:, m * P:(m + 1) * P],
                            qt4[:, hh * SG + off:(hh + 1) * SG],
                            start=True, stop=True)
                    if off == 0:
                        nc.scalar.activation(ex[:], stp[:], Exp)
                    else:
                        nc.scalar.activation(ex3[:, :, off:], stp3[:, :, off:], Exp)
                    if j >= 0:
                        nc.vector.tensor_mul(ex3[:, :, off:off + P],
                                             ex3[:, :, off:off + P], pat3)
                    if m == 0:
                        nc.vector.tensor_copy(exs3[:, 2 * p:2 * p + 2, :], ex3)
                    elif off == 0:
                        nc.vector.tensor_add(exs3[:, 2 * p:2 * p + 2, :],
                                             exs3[:, 2 * p:2 * p + 2, :], ex3)
                    else:
                        nc.vector.tensor_add(exs3[:, 2 * p:2 * p + 2, off:],
                                             exs3[:, 2 * p:2 * p + 2, off:],
                                             ex3[:, :, off:])
                    if cq:
                        emit_piece(True)
                    for h in range(2):
                        nc.tensor.matmul(
                            cacc[p][:, h * SG + off:(h + 1) * SG],
                            v_sb[:, m * HD:(m + 1) * HD],
                            ex[:, h * SG + off:(h + 1) * SG],
                            start=(m == 0), stop=last)

            # ---- finalize: broadcast denominators, reciprocal, scale ----
            nc.vector.tensor_copy(exsum_bf[:], exsum[:])
            for p in range(2):
                bcd = ps.tile([P, 2 * SG], f32, tag="stp1", bufs=1, name="bcd")
                for h in range(2):
                    nc.tensor.matmul(bcd[:, h * SG:(h + 1) * SG], onesm[:],
                                     exsum_bf[:, (2 * p + h) * SG:(2 * p + h + 1) * SG],
                                     start=True, stop=True)
                inv = rp.tile([P, 2 * SG], f32, tag="inv", bufs=1)
                nc.vector.reciprocal_approx_fast(inv[:], bcd[:])
                inv3 = inv[:].rearrange("p (h c) -> p h c", h=2)
                ca3 = cacc[p][:].rearrange("p (h c) -> p h c", h=2)
                nc.vector.tensor_mul(ctx3[:, 2 * p:2 * p + 2, gsl], ca3, inv3)
            cq.extend((mt, ep) for mt in range(4 * G, 4 * G + 4)
                      for ep in range(D // SG // 2))
            if G == 0:
                pending_rope()

        # ---- tail: remaining output-projection pieces ----
        while cq:
            emit_piece(False)

        ab_psum.__exit__(None, None, None)

    nc.compile()
    return nc


def _host_prep(x, wq, wk, wv, wo, freqs_cos, freqs_sin):
    """Build per-core input maps (all layouts pre-tiled for contiguous DMA)."""
    from ml_dtypes import bfloat16
    x = np.ascontiguousarray(np.asarray(x, dtype=np.float32).reshape(S, D))
    wq = np.asarray(wq, dtype=np.float32)
    wk = np.asarray(wk, dtype=np.float32)
    wv = np.asarray(wv, dtype=np.float32)
    wo = np.asarray(wo, dtype=np.float32)

    perm = np.concatenate([np.arange(0, HD, 2), np.arange(1, HD, 2)])
    scale = 1.0 / math.sqrt(HD)
    wq_p = (wq.reshape(D, N_HEADS, HD)[:, :, perm] * scale).astype(np.float32)
    wk_p = wk.reshape(D, N_KV, HD)[:, :, perm]

    # xT stream: xt[p, (G, k, c)] = x[G*SG + c, k*P + p]
    xt = np.ascontiguousarray(
        x.T.reshape(KT, P, NG, SG).transpose(1, 2, 0, 3)
        .reshape(P, NG * KT * SG)).astype(bfloat16)
    fc = np.asarray(freqs_cos, np.float32).T   # [64, S]
    fs = np.asarray(freqs_sin, np.float32).T
    ft = fs / fc                               # tangent (no cancellation)
    ccp = np.ascontiguousarray(np.concatenate([fc, fc], axis=0))  # [c;c]
    ttp = np.ascontiguousarray(np.concatenate([ft, ft], axis=0))  # [t;t]
    # causal triangle pattern for the diagonal 128x128 block, 2 heads wide
    tri = (np.arange(P)[None, :] >= np.arange(P)[:, None]).astype(np.float32)
    pat = np.ascontiguousarray(
        np.broadcast_to(tri[:, None, :], (P, 2, P)).reshape(P, 2 * P)
    ).astype(bfloat16)

    in_maps = []
    for c in range(N_CORES):
        wq_c = wq_p[:, 4 * c:4 * c + 4, :].reshape(D, NH_LOC * HD)
        wq_l = np.ascontiguousarray(
            wq_c.reshape(KT, P, NH_LOC * HD).transpose(1, 0, 2)
            .reshape(P, KT * NH_LOC * HD)).astype(bfloat16)
        wk_c = wk_p[:, c, :]
        wk_l = np.ascontiguousarray(
            wk_c.reshape(KT, P, HD).transpose(1, 0, 2).reshape(P, KT * HD))
        wv_c = wv.reshape(D, N_KV, HD)[:, c, :]
        wv_l = np.ascontiguousarray(
            wv_c.reshape(KT, P, HD).transpose(1, 0, 2).reshape(P, KT * HD))
        wo_c = wo[4 * c * HD:(4 * c + 4) * HD, :]       # [512, D]
        # [P, n, kk, 512]: per dim-group n, the 4 head-chunk tiles adjacent
        wo_l = np.ascontiguousarray(
            wo_c.reshape(NH_LOC, P, D // SG, SG).transpose(1, 2, 0, 3)
            .reshape(P, (D // SG) * NH_LOC * SG))
        in_maps.append({"xt": xt, "wq": wq_l,
                        "wk": wk_l.astype(bfloat16),
                        "wv": wv_l.astype(bfloat16),
                        "wo": wo_l.astype(bfloat16),
                        "ccp": ccp, "ttp": ttp, "pat": pat})
    return in_maps


def _run(x, wq, wk, wv, wo, freqs_cos, freqs_sin, mask, start_pos, trace=False):
    assert int(start_pos) == 0

    if "nc" not in _CACHE:
        _CACHE["nc"] = _build_program()
    nc = _CACHE["nc"]

    in_maps = _host_prep(x, wq, wk, wv, wo, freqs_cos, freqs_sin)

    from concourse.bass_utils import run_bass_kernel_spmd
    res = run_bass_kernel_spmd(nc, in_maps, list(range(N_CORES)), trace=trace)
    out = np.zeros((S, D), dtype=np.float32)
    for c in range(N_CORES):
        out += res.results[c]["out"].astype(np.float32)
    return out.reshape(1, S, D), res


def kernel(x, wq, wk, wv, wo, freqs_cos, freqs_sin, mask, start_pos):
    out, _ = _run(x, wq, wk, wv, wo, freqs_cos, freqs_sin, mask, start_pos)
    return out


# revision 29
# speedup vs baseline: 1.1701x; 1.1701x over previous
"""Trainium2 Bass kernel for Llama-style GQA attention (B=1, S=2048, D=4096,
32 Q heads / 8 KV heads, head_dim 128, RoPE, causal mask).

Sharding: 8-way tensor-parallel over heads. Core c computes Q heads 4c..4c+3
and KV head c end-to-end (projections + RoPE + attention + its rows of wo),
producing a partial [S, D] output in bf16; the host sums the 8 partials (the
all-reduce of the row-parallel wo).

v3 design (all matmul operands bf16, PSUM accumulation fp32):
  - All four 512-wide projection groups run first; each group's RoPE (DVE)
    overlaps the next group's projections (PE), so the PE never waits on
    RoPE except ~2us at each boundary.  PSUM tiles are released after a
    SINGLE full-width read: q*sin is recomputed from q*cos via a tangent
    table (q2 = p1 * tan, numerically safe since no cancellation).
  - RoPE's even/odd interleave is folded into a column permutation of
    wq/wk; cos/tan tables are stacked [c;c] and duplicated per head-pair
    so RoPE runs 1024 wide for q.
  - Scores transposed: ST[sk, sq] = K @ Q^T.  Two single-bank score
    matmuls per (m, head-pair) share an adjacent-bank PSUM tile so exp is
    ONE wide ACT instruction.  Causal trimming: score/exp/PV/exsum touch
    only valid columns; the diagonal 128x128 triangle is zeroed with one
    multiplicative bf16 pattern.
  - Softmax denominators: DVE accumulates exp tiles (fp32), a ones-matrix
    matmul broadcasts the partition-sum to all 128 partitions, one
    custom-DVE reciprocal_approx and a multiply normalize PSUM context
    into bf16 ctx.  No serial [1,N] reciprocals, no per-tile denominator
    matmuls.
  - Output projection: wo resident, kk-outer 512-wide matmuls, po drained
    by split DVE/ACT copies to bf16 and DMA'd out.
"""

import math
import numpy as np

P = 128          # SBUF partitions / head_dim / tile edge
S = 2048         # sequence length
D = 4096         # model dim
HD = 128         # head dim
N_HEADS = 32
N_KV = 8
N_CORES = 8
NH_LOC = N_HEADS // N_CORES   # 4 local Q heads
SG = 512         # query-group width
NG = S // SG     # 4 q-position groups
KT = D // P      # 32 contraction tiles for projections
NSK = S // P     # 16 key tiles

_CACHE = {}


def _build_program():
    import concourse.tile as tile
    from concourse import bacc, mybir
    from concourse.masks import make_identity
    from contextlib import ExitStack

    f32 = mybir.dt.float32
    bf16 = mybir.dt.bfloat16
    Exp = mybir.ActivationFunctionType.Exp

    nc = bacc.Bacc()
    xt_d = nc.dram_tensor("xt", [P, NG * KT * SG], bf16, kind="ExternalInput")
    wq_d = nc.dram_tensor("wq", [P, KT * NH_LOC * HD], bf16, kind="ExternalInput")
    wk_d = nc.dram_tensor("wk", [P, KT * HD], bf16, kind="ExternalInput")
    wv_d = nc.dram_tensor("wv", [P, KT * HD], bf16, kind="ExternalInput")
    wo_d = nc.dram_tensor("wo", [P, (D // SG) * NH_LOC * SG], bf16,
                          kind="ExternalInput")
    ccp_d = nc.dram_tensor("ccp", [P, S], f32, kind="ExternalInput")
    ttp_d = nc.dram_tensor("ttp", [P, S], f32, kind="ExternalInput")
    pat_d = nc.dram_tensor("pat", [P, 2 * P], bf16, kind="ExternalInput")
    out_d = nc.dram_tensor("out", [S, D], bf16, kind="ExternalOutput")

    with ExitStack() as ctx:
        tc = ctx.enter_context(tile.TileContext(nc))
        consts = ctx.enter_context(tc.tile_pool(name="consts", bufs=1))
        kv = ctx.enter_context(tc.tile_pool(name="kv", bufs=1))
        xp = ctx.enter_context(tc.tile_pool(name="xp", bufs=5))
        qp = ctx.enter_context(tc.tile_pool(name="qp", bufs=4))
        rp = ctx.enter_context(tc.tile_pool(name="rp", bufs=2))
        ep = ctx.enter_context(tc.tile_pool(name="ep", bufs=4))
        obp = ctx.enter_context(tc.tile_pool(name="obp", bufs=3))

        # ---- resident weights / constants ----
        # staged in k-rounds so phase A(0) can start after ~1.5MB arrives;
        # wo is emitted after A(0) so it never starves the xt stream.
        wq_sb = consts.tile([P, KT * NH_LOC * HD], bf16)
        wk_sb = consts.tile([P, KT * HD], bf16)
        wv_sb = consts.tile([P, KT * HD], bf16)
        qtr = KT * NH_LOC * HD // 8
        ktr = KT * HD // 8
        for i in range(8):
            nc.scalar.dma_start(wq_sb[:, i * qtr:(i + 1) * qtr],
                                wq_d[:, i * qtr:(i + 1) * qtr])
            nc.scalar.dma_start(wk_sb[:, i * ktr:(i + 1) * ktr],
                                wk_d[:, i * ktr:(i + 1) * ktr])
            nc.scalar.dma_start(wv_sb[:, i * ktr:(i + 1) * ktr],
                                wv_d[:, i * ktr:(i + 1) * ktr])
        ccp_sb = consts.tile([P, S], f32)
        ttp_sb = consts.tile([P, S], f32)
        nc.scalar.dma_start(ccp_sb[:], ccp_d[:, :])
        nc.scalar.dma_start(ttp_sb[:], ttp_d[:, :])
        pat_sb = consts.tile([P, 2 * P], bf16)
        nc.scalar.dma_start(pat_sb[:], pat_d[:, :])

        ones_f = consts.tile([P, P], f32)
        nc.vector.memset(ones_f[:], 1.0)
        onesm = consts.tile([P, P], bf16)
        nc.vector.tensor_copy(onesm[:], ones_f[:])
        ident = consts.tile([P, P], f32)
        make_identity(nc, ident[:])
        identb = consts.tile([P, P], bf16)
        nc.vector.tensor_copy(identb[:], ident[:])

        # ---- persistent per-sequence state ----
        kT_sb = kv.tile([P, S], bf16)                 # [hd', sk]
        v_sb = kv.tile([P, S], bf16)                  # [sk%P, (sk//P)*HD+hd]
        ctx_sb = kv.tile([P, NH_LOC * S], bf16)       # [hd, h*S + sq]
        exsum = kv.tile([P, NH_LOC * SG], f32)        # [sk', h*SG + sq-in-G]
        exsum_bf = kv.tile([P, NH_LOC * SG], bf16)

        ctx3 = ctx_sb[:].rearrange("p (h c) -> p h c", h=NH_LOC)
        exs3 = exsum[:].rearrange("p (h c) -> p h c", h=NH_LOC)
        pat3 = pat_sb[:].rearrange("p (h c) -> p h c", h=2)

        ab_psum = tc.tile_pool(name="ps", bufs=2, space="PSUM")
        ps = ab_psum.__enter__()

        wo_sb = consts.tile([P, (D // SG) * NH_LOC * SG], bf16)

        # ================= phase A: projections + RoPE, all groups =========
        qts = []
        for G in range(NG):
            pq01 = ps.tile([P, 2 * SG], f32, tag="acc2", bufs=2, name="pq01")
            pq23 = ps.tile([P, 2 * SG], f32, tag="acc2", bufs=2, name="pq23")
            pkv = ps.tile([P, 2 * SG], f32, tag="stp1", bufs=1, name="pkv")
            for k4 in range(KT // 4):
                xt2 = xp.tile([P, 4 * SG], bf16, tag="xt", bufs=3, name="xt")
                blk = (G * KT + 4 * k4) * SG
                if G == 0 and k4 == 0:
                    for q in range(4):
                        nc.gpsimd.dma_start(xt2[:, q * SG:(q + 1) * SG],
                                            xt_d[:, blk + q * SG:blk + (q + 1) * SG])
                else:
                    nc.gpsimd.dma_start(xt2[:], xt_d[:, blk:blk + 4 * SG])
                for kk in (0, 1, 2, 3):
                    k = 4 * k4 + kk
                    xt = xt2[:, kk * SG:(kk + 1) * SG]
                    st_k, sp_k = (k == 0), (k == KT - 1)
                    for l in range(2):
                        nc.tensor.matmul(
                            pq01[:, l * SG:(l + 1) * SG],
                            wq_sb[:, k * SG + l * HD:k * SG + (l + 1) * HD],
                            xt, start=st_k, stop=sp_k)
                    for l in range(2, 4):
                        nc.tensor.matmul(
                            pq23[:, (l - 2) * SG:(l - 1) * SG],
                            wq_sb[:, k * SG + l * HD:k * SG + (l + 1) * HD],
                            xt, start=st_k, stop=sp_k)
                    nc.tensor.matmul(pkv[:, 0:SG],
                                     wk_sb[:, k * HD:(k + 1) * HD], xt,
                                     start=st_k, stop=sp_k)
                    nc.tensor.matmul(pkv[:, SG:2 * SG],
                                     wv_sb[:, k * HD:(k + 1) * HD], xt,
                                     start=st_k, stop=sp_k)

            if G == 0:
                # wo loads (4MB) start only now: keeps startup DMA light
                wtr = (D // SG) * NH_LOC * SG // 4
                for i in range(4):
                    nc.scalar.dma_start(wo_sb[:, i * wtr:(i + 1) * wtr],
                                        wo_d[:, i * wtr:(i + 1) * wtr])

            # ---- RoPE ----  (rows 0:64 "real" tr, 64:128 "imag" ti)
            # p1 = src*[c;c]  (the ONLY psum read -> frees the bank fast)
            # q2[0:64]=p1[64:]*tan[64:]=ti*s ; q2[64:]=p1[0:64]*tan[0:64]=tr*s
            # top = tr*c - ti*s ; bot = tr*s + ti*c   (all SBUF, base-aligned)
            qt4 = qp.tile([P, NH_LOC * SG], bf16, tag="qT", bufs=4, name="qT")
            qts.append(qt4)
            cpw = ccp_sb[:, G * SG:(G + 1) * SG]   # [c;c] for this window
            tpw = ttp_sb[:, G * SG:(G + 1) * SG]

            vt = rp.tile([P, SG], bf16, tag="vt", bufs=1)
            nc.vector.tensor_copy(vt[:], pkv[:, SG:2 * SG])
            p1k = rp.tile([P, SG], bf16, tag="p1k", bufs=1)
            nc.vector.tensor_mul(p1k[:], pkv[:, 0:SG], cpw)
            p1q = []
            for pq in (pq01, pq23):
                p1 = rp.tile([P, 2 * SG], bf16, tag="p1", bufs=2)
                nc.vector.tensor_mul(p1[:, 0:SG], pq[:, 0:SG], cpw)
                nc.vector.tensor_mul(p1[:, SG:2 * SG], pq[:, SG:2 * SG], cpw)
                p1q.append(p1)

            # v transpose on PE while DVE continues RoPE
            ptr4 = ps.tile([P, SG], bf16, tag="po1", bufs=2, name="ptr4")
            for j in range(4):
                nc.tensor.transpose(ptr4[:, j * P:(j + 1) * P],
                                    vt[:, j * P:(j + 1) * P], identb[:])

            def rope_finish(G=G, qt4=qt4, p1q=p1q, p1k=p1k, ptr4=ptr4,
                            tpw=tpw):
                for i, p1 in enumerate(p1q):
                    q2 = rp.tile([P, 2 * SG], bf16, tag="q2", bufs=1)
                    for hs in (slice(0, SG), slice(SG, 2 * SG)):
                        nc.vector.tensor_mul(q2[0:64, hs], p1[64:128, hs],
                                             tpw[64:128, :])
                        nc.vector.tensor_mul(q2[64:128, hs], p1[0:64, hs],
                                             tpw[0:64, :])
                    dst = qt4[:, 2 * i * SG:(2 * i + 2) * SG]
                    nc.vector.tensor_sub(dst[0:64, :], p1[0:64, :], q2[0:64, :])
                    nc.vector.tensor_add(dst[64:128, :], q2[64:128, :],
                                         p1[64:128, :])
                q2k = rp.tile([P, SG], bf16, tag="q2k", bufs=1)
                nc.vector.tensor_mul(q2k[0:64, :], p1k[64:128, :], tpw[64:128, :])
                nc.vector.tensor_mul(q2k[64:128, :], p1k[0:64, :], tpw[0:64, :])
                gsl = slice(G * SG, (G + 1) * SG)
                nc.vector.tensor_sub(kT_sb[0:64, gsl], p1k[0:64, :], q2k[0:64, :])
                nc.vector.tensor_add(kT_sb[64:128, gsl], q2k[64:128, :],
                                     p1k[64:128, :])
                nc.vector.tensor_copy(v_sb[:, 4 * G * HD:(4 * G + 4) * HD],
                                      ptr4[:])

            if G < NG - 1:
                rope_finish()
            else:
                pending_rope = rope_finish   # emit after B(0): keeps B(0)'s
                                             # patmul/exsum off the DVE queue

        # ================= phase B/C fused: attention + output proj ========
        # phase C is decomposed into single-bank pieces (4 matmuls each);
        # pieces for already-finalized groups are interleaved into phase B's
        # exp-bound iterations so the PE fills ACT-wait windows.
        cq = []

        def emit_piece(in_b):
            mt, e = cq.pop(0)
            po = ps.tile([P, SG], f32, tag="po1", bufs=2, name="po")
            for kk in range(NH_LOC):
                nc.tensor.matmul(po[:],
                                 ctx3[:, kk, mt * P:(mt + 1) * P],
                                 wo_sb[:, (e * NH_LOC + kk) * SG:
                                       (e * NH_LOC + kk + 1) * SG],
                                 start=(kk == 0), stop=(kk == NH_LOC - 1))
            ob = obp.tile([P, SG], bf16, tag="ob", bufs=6)
            if in_b or (len(cq) % 2 == 0):
                nc.vector.tensor_copy(ob[:], po[:])
            else:
                nc.scalar.copy(ob[:], po[:])
            nc.sync.dma_start(out_d[mt * P:(mt + 1) * P, e * SG:(e + 1) * SG],
                              ob[:])

        for G in range(NG):
            gsl = slice(G * SG, (G + 1) * SG)
            n_sk = 4 * (G + 1)
            qt4 = qts[G]
            cacc = [ps.tile([P, 2 * SG], f32, tag="acc2", bufs=2, name=f"cacc{p}")
                    for p in range(2)]
            for m in range(n_sk):
                j = m - 4 * G
                off = max(0, j) * P
                last = (m == n_sk - 1)
                for p in range(2):
                    stp = ps.tile([P, 2 * SG], f32, tag="stp1", bufs=1, name="stp")
                    stp3 = stp[:].rearrange("p (h c) -> p h c", h=2)
                    ex = ep.tile([P, 2 * SG], bf16, tag="ex", bufs=4, name="ex")
                    ex3 = ex[:].rearrange("p (h c) -> p h c", h=2)
                    for h in range(2):
                        hh = 2 * p + h
                        nc.tensor.matmul(
                            stp[:, h * SG + off:(h + 1) * SG],
                            kT_sb[:, m * P:(m + 1) * P],
                            qt4[:, hh * SG + off:(hh + 1) * SG],
                            start=True, stop=True)
                    if off == 0:
                        nc.scalar.activation(ex[:], stp[:], Exp)
                    else:
                        nc.scalar.activation(ex3[:, :, off:], stp3[:, :, off:], Exp)
                    if j >= 0:
                        nc.vector.tensor_mul(ex3[:, :, off:off + P],
                                             ex3[:, :, off:off + P], pat3)
                    if m == 0:
                        nc.vector.tensor_copy(exs3[:, 2 * p:2 * p + 2, :], ex3)
                    elif off == 0:
                        nc.vector.tensor_add(exs3[:, 2 * p:2 * p + 2, :],
                                             exs3[:, 2 * p:2 * p + 2, :], ex3)
                    else:
                        nc.vector.tensor_add(exs3[:, 2 * p:2 * p + 2, off:],
                                             exs3[:, 2 * p:2 * p + 2, off:],
                                             ex3[:, :, off:])
                    if cq:
                        emit_piece(True)
                    for h in range(2):
                        nc.tensor.matmul(
                            cacc[p][:, h * SG + off:(h + 1) * SG],
                            v_sb[:, m * HD:(m + 1) * HD],
                            ex[:, h * SG + off:(h + 1) * SG],
                            start=(m == 0), stop=last)

            # ---- finalize: broadcast denominators, reciprocal, scale ----
            nc.vector.tensor_copy(exsum_bf[:], exsum[:])
            for p in range(2):
                bcd = ps.tile([P, 2 * SG], f32, tag="stp1", bufs=1, name="bcd")
                for h in range(2):
                    nc.tensor.matmul(bcd[:, h * SG:(h + 1) * SG], onesm[:],
                                     exsum_bf[:, (2 * p + h) * SG:(2 * p + h + 1) * SG],
                                     start=True, stop=True)
                inv = rp.tile([P, 2 * SG], f32, tag="inv", bufs=1)
                nc.vector.reciprocal_approx_fast(inv[:], bcd[:])
                inv3 = inv[:].rearrange("p (h c) -> p h c", h=2)
                ca3 = cacc[p][:].rearrange("p (h c) -> p h c", h=2)
                nc.vector.tensor_mul(ctx3[:, 2 * p:2 * p + 2, gsl], ca3, inv3)
            cq.extend((mt, e) for mt in range(4 * G, 4 * G + 4)
                      for e in range(D // SG))
            if G == 0:
                pending_rope()

        # ---- tail: remaining output-projection pieces ----
        while cq:
            emit_piece(False)

        ab_psum.__exit__(None, None, None)

    nc.compile()
    return nc


def _host_prep(x, wq, wk, wv, wo, freqs_cos, freqs_sin):
    """Build per-core input maps (all layouts pre-tiled for contiguous DMA)."""
    from ml_dtypes import bfloat16
    x = np.ascontiguousarray(np.asarray(x, dtype=np.float32).reshape(S, D))
    wq = np.asarray(wq, dtype=np.float32)
    wk = np.asarray(wk, dtype=np.float32)
    wv = np.asarray(wv, dtype=np.float32)
    wo = np.asarray(wo, dtype=np.float32)

    perm = np.concatenate([np.arange(0, HD, 2), np.arange(1, HD, 2)])
    scale = 1.0 / math.sqrt(HD)
    wq_p = (wq.reshape(D, N_HEADS, HD)[:, :, perm] * scale).astype(np.float32)
    wk_p = wk.reshape(D, N_KV, HD)[:, :, perm]

    # xT stream: xt[p, (G, k, c)] = x[G*SG + c, k*P + p]
    xt = np.ascontiguousarray(
        x.T.reshape(KT, P, NG, SG).transpose(1, 2, 0, 3)
        .reshape(P, NG * KT * SG)).astype(bfloat16)
    fc = np.asarray(freqs_cos, np.float32).T   # [64, S]
    fs = np.asarray(freqs_sin, np.float32).T
    ft = fs / fc                               # tangent (no cancellation)
    ccp = np.ascontiguousarray(np.concatenate([fc, fc], axis=0))  # [c;c]
    ttp = np.ascontiguousarray(np.concatenate([ft, ft], axis=0))  # [t;t]
    # causal triangle pattern for the diagonal 128x128 block, 2 heads wide
    tri = (np.arange(P)[None, :] >= np.arange(P)[:, None]).astype(np.float32)
    pat = np.ascontiguousarray(
        np.broadcast_to(tri[:, None, :], (P, 2, P)).reshape(P, 2 * P)
    ).astype(bfloat16)

    in_maps = []
    for c in range(N_CORES):
        wq_c = wq_p[:, 4 * c:4 * c + 4, :].reshape(D, NH_LOC * HD)
        wq_l = np.ascontiguousarray(
            wq_c.reshape(KT, P, NH_LOC * HD).transpose(1, 0, 2)
            .reshape(P, KT * NH_LOC * HD)).astype(bfloat16)
        wk_c = wk_p[:, c, :]
        wk_l = np.ascontiguousarray(
            wk_c.reshape(KT, P, HD).transpose(1, 0, 2).reshape(P, KT * HD))
        wv_c = wv.reshape(D, N_KV, HD)[:, c, :]
        wv_l = np.ascontiguousarray(
            wv_c.reshape(KT, P, HD).transpose(1, 0, 2).reshape(P, KT * HD))
        wo_c = wo[4 * c * HD:(4 * c + 4) * HD, :]       # [512, D]
        # [P, n, kk, 512]: per dim-group n, the 4 head-chunk tiles adjacent
        wo_l = np.ascontiguousarray(
            wo_c.reshape(NH_LOC, P, D // SG, SG).transpose(1, 2, 0, 3)
            .reshape(P, (D // SG) * NH_LOC * SG))
        in_maps.append({"xt": xt, "wq": wq_l,
                        "wk": wk_l.astype(bfloat16),
                        "wv": wv_l.astype(bfloat16),
                        "wo": wo_l.astype(bfloat16),
                        "ccp": ccp, "ttp": ttp, "pat": pat})
    return in_maps


def _run(x, wq, wk, wv, wo, freqs_cos, freqs_sin, mask, start_pos, trace=False):
    assert int(start_pos) == 0

    if "nc" not in _CACHE:
        _CACHE["nc"] = _build_program()
    nc = _CACHE["nc"]

    in_maps = _host_prep(x, wq, wk, wv, wo, freqs_cos, freqs_sin)

    from concourse.bass_utils import run_bass_kernel_spmd
    res = run_bass_kernel_spmd(nc, in_maps, list(range(N_CORES)), trace=trace)
    out = np.zeros((S, D), dtype=np.float32)
    for c in range(N_CORES):
        out += res.results[c]["out"].astype(np.float32)
    return out.reshape(1, S, D), res


def kernel(x, wq, wk, wv, wo, freqs_cos, freqs_sin, mask, start_pos):
    out, _ = _run(x, wq, wk, wv, wo, freqs_cos, freqs_sin, mask, start_pos)
    return out
